# revision 19
# baseline (speedup 1.0000x reference)
"""Trainium2 Bass kernel for nn_AttnBlock (per-pixel qk attention block).

Reference computation (per batch b):
  q = x @ wq.T ; k = x @ wk.T ; v = x @ wv.T          # [H*W, 512], heads n=8, d=64
  s[n, p]    = sum_d q[p, n*64+d] * k[p, n*64+d]      # per-pixel dot product
  w[n, h, :] = softmax(s[n, h, :] * d**-0.5)          # softmax over W axis (32)
  vsum[n, p] = sum_d v[p, n*64+d]
  out[b, n, hw, xy] = w[n, hw] * vsum[n, xy]          # outer product per batch

Sharding: data-parallel over batch: core b handles batch b (8 cores, B=8).

The kernel is output-write bound: fp16 output (host upcasts; rel err ~1e-3
vs the 2e-2 gate) => 16 MB/core written + 2 MB read at the ~358 GB/s per-NC
HBM limit => ~52.7 us DMA floor. The design keeps the DMA queue busy
end-to-end: the input stream is packed into 7 DMAs (aux+pair-0 weights
merged into one "pre" tensor so x^T starts immediately), and the first
output tiles are ready shortly after the last input lands.

v6 implementation notes:
- host does layout prep only (transposes/reshapes/casts; the only
  arithmetic is the head-block sum of wv rows, O(dim^2)).
- PE warm-up: dummy matmuls (own PSUM bank) from t~0 so the tensor engine
  reaches full clock (cost model: 2.4 GHz after 3 us continuous busy)
  before the x-gated burst; spares also fill x-chunk stall gaps.
- q/k PSUM never drains: sprod = q*k reads both PSUM banks directly.
- pair 0 runs a fine-grained high-priority softmax (per-128-col chunk
  exp/reduce/recip/mult/PE-transpose) so the first 1 MB DMA starts ~5 us
  after the last x chunk; pairs 1-3 use half-granularity ops.
- production per head: PE selector-matmul broadcast of vsum (drains on
  ACT), 8 outer-product tiles [128, 1024] via DVE tensor_scalar fp16
  (4x mode, 327 ns; 2 of 8 on ACT for heads 1-7), then the head's 2 MB
  DMA (head 0: 2x 1 MB to open the chain early).
"""

import numpy as np

import concourse.bass as bass
import concourse.mybir as mybir
import concourse.tile as tile
from concourse import bacc
from concourse.bass_utils import run_bass_kernel_spmd

F32 = mybir.dt.float32
F16 = mybir.dt.float16

B, HW, DIM = 8, 1024, 512
N_HEADS, D_HEAD = 8, 64
N_CORES = 8
SCALE = float(D_HEAD) ** -0.5

QK_DT = F16
QK_NP = np.float16
OUT_DT = F16

N_WARMUP = 16  # dummy PE matmuls: ~3 us ramp + spares for x-stall gaps


def build_program(loop_iters=None):
    """loop_iters: if set, wrap the whole kernel body in a tc.For_i hardware
    loop (benchmarking only -- one NEFF executes the body N times)."""
    # Bacc (not raw Bass): its compile() runs move_matmul_waits_to_ldweights,
    # without which any matmul with >1 semaphore wait fails walrus codegen.
    nc = bacc.Bacc(None)

    xt_d = nc.declare_dram_parameter("xt", [DIM, HW], QK_DT, isOutput=False)
    # pre = aux (wv_sum + ind2) and the pair-0 wq/wk slices, one DMA
    pre_d = nc.declare_dram_parameter("pre", [128, 1058], QK_DT, isOutput=False)
    wrest_d = nc.declare_dram_parameter(
        "wrest", [128, 2, 3, 4, 128], QK_DT, isOutput=False
    )
    aux2_d = nc.declare_dram_parameter("aux2", [8, 1026], QK_DT, isOutput=False)
    y_d = nc.declare_dram_parameter("y", [N_HEADS, HW, HW], OUT_DT, isOutput=True)

    with tile.TileContext(nc) as tc:
        with (
            tc.tile_pool(name="singles", bufs=1) as singles,
            tc.tile_pool(name="sprod", bufs=2) as sprodp,
            tc.tile_pool(name="smax", bufs=2) as smaxp,
            tc.tile_pool(name="wt", bufs=2) as wtp,
            tc.tile_pool(name="bc", bufs=2) as bcp,
            tc.tile_pool(name="prod", bufs=2) as prodp,
            tc.tile_pool(name="warm_ps", bufs=1, space="PSUM") as warm_ps,
            tc.tile_pool(name="tp_ps", bufs=1, space="PSUM") as tp_ps,
            tc.tile_pool(name="qk_ps", bufs=3, space="PSUM") as qk_ps,
            tc.tile_pool(name="s_ps", bufs=1, space="PSUM") as s_ps,
            tc.tile_pool(name="v_ps", bufs=1, space="PSUM") as v_ps,
        ):
            def emit_body():
                # ---- PE warm-up tile (dummy matmuls emitted last, so they
                # have the lowest priority and only fill idle PE slots) ------
                wm = singles.tile([128, 512], QK_DT, name="wm")
                nc.gpsimd.memset(wm, 0.0)

                # ---- loads: one FIFO queue (sync HWDGE), priority order ----
                pre_sb = singles.tile([128, 1058], QK_DT)
                nc.sync.dma_start(out=pre_sb, in_=pre_d[:])
                wvt_sb = pre_sb[:, 0:32].rearrange("p (k n) -> p k n", k=4)
                ind2_sb = pre_sb[:, 32:34]
                wq0_sb = pre_sb[:, 34:546].rearrange("p (k o) -> p k o", k=4)
                wk0_sb = pre_sb[:, 546:1058].rearrange("p (k o) -> p k o", k=4)

                xT = []
                xv = xt_d[:].rearrange("(k p) xy -> p k xy", p=128)
                for ki in range(4):
                    xt_t = singles.tile([128, HW], QK_DT, name=f"xT{ki}")
                    nc.sync.dma_start(out=xt_t, in_=xv[:, ki, :])
                    xT.append(xt_t)

                aux2_sb = singles.tile([8, 1026], QK_DT)
                nc.sync.dma_start(out=aux2_sb, in_=aux2_d[:])
                sel_sb = aux2_sb[:, 0:1024]
                ident2_sb = aux2_sb[0:2, 1024:1026]

                wq_sb = singles.tile([128, 3, 4, 128], QK_DT)  # pairs 1-3
                wk_sb = singles.tile([128, 3, 4, 128], QK_DT)
                nc.sync.dma_start(out=wq_sb, in_=wrest_d[:, 0])
                nc.sync.dma_start(out=wk_sb, in_=wrest_d[:, 1])

                def wslice(w0_sb, wr_sb, ti, ki):
                    if ti == 0:
                        return w0_sb[:, ki, :]
                    return wr_sb[:, ti - 1, ki, :]

                def emit_qk_half(ti, nj):
                    """q/k matmul groups for pixel-half nj of pair ti."""
                    qps = qk_ps.tile([128, 512], F32, tag="qk", name="qps")
                    kps = qk_ps.tile([128, 512], F32, tag="qk", name="kps")
                    for ps, w0, wr in ((qps, wq0_sb, wq_sb), (kps, wk0_sb, wk_sb)):
                        for ki in range(4):
                            nc.tensor.matmul(
                                ps,
                                wslice(w0, wr, ti, ki),
                                xT[ki][:, nj * 512 : (nj + 1) * 512],
                                start=(ki == 0),
                                stop=(ki == 3),
                            )
                    return qps, kps

                def emit_scores_half(qps, kps):
                    """sprod straight from the q/k PSUM banks + score matmul."""
                    sprod = sprodp.tile([128, 512], QK_DT, tag="sp")
                    nc.vector.tensor_tensor(
                        out=sprod, in0=qps, in1=kps, op=mybir.AluOpType.mult,
                    )
                    sps = s_ps.tile([2, 512], F32, tag="s")
                    nc.tensor.matmul(sps, ind2_sb, sprod, start=True, stop=True)
                    return sps

                def emit_softmax_chunk(sps, cloc, ncol):
                    """exp/reduce/recip/mult over sps cols [cloc, cloc+ncol).
                    Returns the w chunk tile [2, ncol] fp16."""
                    csl = slice(cloc, cloc + ncol)
                    e_h = smaxp.tile([2, ncol], QK_DT, tag=f"e{cloc}_{ncol}")
                    nc.scalar.activation(
                        out=e_h, in_=sps[:, csl],
                        func=mybir.ActivationFunctionType.Exp,
                        scale=SCALE,
                    )
                    nh = ncol // 32
                    denom = smaxp.tile([2, nh], QK_DT, tag=f"d{cloc}_{ncol}")
                    with nc.allow_low_precision(reason="fp16 softmax denom"):
                        nc.vector.tensor_reduce(
                            out=denom,
                            in_=e_h.rearrange("p (h w) -> p h w", w=32),
                            axis=mybir.AxisListType.X,
                            op=mybir.AluOpType.add,
                        )
                        rden = smaxp.tile([2, nh], QK_DT, tag=f"r{cloc}_{ncol}")
                        nc.vector.reciprocal(rden, denom)
                    w_h = smaxp.tile([2, ncol], QK_DT, tag=f"w{cloc}_{ncol}")
                    rden_b = bass.AP(
                        tensor=rden.tensor, offset=rden.offset,
                        ap=[*rden.ap, [0, 32]],
                    )
                    nc.vector.tensor_tensor(
                        out=w_h.rearrange("p (h w) -> p h w", w=32),
                        in0=e_h.rearrange("p (h w) -> p h w", w=32),
                        in1=rden_b,
                        op=mybir.AluOpType.mult,
                    )
                    return w_h

                def emit_bcast(head, bcast_t=None):
                    """vsum row -> all partitions via PE selector matmul;
                    PSUM drains on ACT."""
                    if bcast_t is None:
                        bcast_t = bcp.tile([128, HW], QK_DT, tag="bc", name="bc")
                    for nj in range(2):
                        bps = qk_ps.tile([128, 512], F32, tag="qk", name="bps")
                        nc.tensor.matmul(
                            bps,
                            sel_sb[:, head * 128 : (head + 1) * 128],
                            vsum_sb[:, nj * 512 : (nj + 1) * 512],
                            start=True, stop=True,
                        )
                        nc.scalar.copy(bcast_t[:, nj * 512 : (nj + 1) * 512], bps)
                    return bcast_t

                def dma_rows(head, prod_t, j, c0, c1):
                    nc.sync.dma_start(
                        out=y_d[head : head + 1].rearrange(
                            "n (c p) xy -> p n c xy", p=128
                        )[:, :, c0:c1, :],
                        in_=prod_t[:, j : j + 1, c0:c1, :],
                    )

                # ---- pair 0 first: its q/k + scores get high priority so
                # the first output DMA lands as early as possible; vsum sits
                # between pair-0 h0 and h1 on the PE. All pairs share the
                # same half-granularity softmax/production pipeline.
                with tc.high_priority():
                    qk00 = emit_qk_half(0, 0)

                vps = v_ps.tile([N_HEADS, HW], F32)
                vsum_sb = singles.tile([N_HEADS, HW], QK_DT)
                for nj in range(2):
                    for ki in range(4):
                        nc.tensor.matmul(
                            vps[:, nj * 512 : (nj + 1) * 512],
                            wvt_sb[:, ki, :],
                            xT[ki][:, nj * 512 : (nj + 1) * 512],
                            start=(ki == 0),
                            stop=(ki == 3),
                        )

                with tc.high_priority():
                    sps00 = emit_scores_half(*qk00)
                    w_h00 = emit_softmax_chunk(sps00, 0, 512)

                # vsum -> fp16 halves (ACT; gates the selector broadcast)
                for nj in range(2):
                    nc.scalar.copy(
                        vsum_sb[:, nj * 512 : (nj + 1) * 512],
                        vps[:, nj * 512 : (nj + 1) * 512],
                    )

                # head-0 broadcast early: only needs vsum + sel
                with tc.high_priority():
                    bcast0 = emit_bcast(0)

                for ti in range(4):
                    wt_sb = wtp.tile([128, 8, 2], F32, tag="wt", name="wt")
                    tp = tp_ps.tile([128, 16], QK_DT, tag="tp", name="tp")

                    def transpose4(w_h, cbase):
                        for cr in range(4):
                            nc.tensor.transpose(
                                tp[:, (cbase + cr) * 2 : (cbase + cr + 1) * 2],
                                w_h[:, cr * 128 : (cr + 1) * 128],
                                ident2_sb,
                            )

                    def drain_wt(c0, c1):
                        nc.vector.tensor_copy(
                            wt_sb[:, c0:c1, :],
                            tp[:, c0 * 2 : c1 * 2].rearrange(
                                "p (c n) -> p c n", c=c1 - c0
                            ),
                        )

                    prod_t = prodp.tile([128, 2, 8, HW], OUT_DT, tag="pr", name="pr")

                    def produce(j, cj, bcast_t):
                        head = 2 * ti + j
                        if cj in (2, 5) and head > 0:
                            nc.scalar.activation(
                                out=prod_t[:, j, cj, :], in_=bcast_t,
                                func=mybir.ActivationFunctionType.Copy,
                                scale=wt_sb[:, cj, j : j + 1],
                            )
                        else:
                            nc.vector.tensor_scalar_mul(
                                prod_t[:, j, cj, :], bcast_t,
                                wt_sb[:, cj, j : j + 1],
                            )

                    if ti == 0:
                        # h0's wt columns drain as soon as its 4 transposes
                        # are done; the first 1 MB DMA goes before h1's
                        # softmax completes
                        with tc.high_priority():
                            transpose4(w_h00, 0)
                            drain_wt(0, 4)
                            for cj in range(4):
                                produce(0, cj, bcast0)
                            dma_rows(0, prod_t, 0, 0, 4)
                        with tc.high_priority():
                            qps, kps = emit_qk_half(0, 1)
                            sps = emit_scores_half(qps, kps)
                            w_h01 = emit_softmax_chunk(sps, 0, 512)
                            transpose4(w_h01, 4)
                            drain_wt(4, 8)
                            for cj in range(4, 8):
                                produce(0, cj, bcast0)
                            dma_rows(0, prod_t, 0, 4, 8)
                            bcast_t = emit_bcast(1)
                            for cj in range(8):
                                produce(1, cj, bcast_t)
                            dma_rows(1, prod_t, 1, 0, 8)
                        continue

                    w_halves = []
                    for nj in range(2):
                        qps, kps = emit_qk_half(ti, nj)
                        sps = emit_scores_half(qps, kps)
                        w_halves.append(emit_softmax_chunk(sps, 0, 512))
                    transpose4(w_halves[0], 0)
                    transpose4(w_halves[1], 4)
                    drain_wt(0, 8)
                    for j in range(2):
                        head = 2 * ti + j
                        bcast_t = emit_bcast(head)
                        for cj in range(8):
                            produce(j, cj, bcast_t)
                        dma_rows(head, prod_t, j, 0, 8)

                # PE warm-up dummies: emitted last => lowest priority, they
                # only run when no real matmul is ready (t~0 and x-stalls)
                for wi in range(N_WARMUP):
                    wps = warm_ps.tile([128, 512], F32, tag="w")
                    nc.tensor.matmul(
                        wps, wm[:, 0:128], wm, start=True, stop=True,
                    )

            if loop_iters:
                with tc.For_i(0, loop_iters, 1):
                    emit_body()
            else:
                emit_body()

    nc.compile()
    return nc


_NC_CACHE = None


def _get_nc():
    global _NC_CACHE
    if _NC_CACHE is None:
        _NC_CACHE = build_program()
    return _NC_CACHE


def make_in_maps(x, wq, wk, wv):
    """Host-side input prep: dtype casts and layout transforms only (transpose,
    reshape, head-block sum of wv -- no x-dependent compute beyond layout),
    plus per-core batch sharding."""
    x = np.ascontiguousarray(np.asarray(x, dtype=np.float32))
    wq = np.asarray(wq, dtype=np.float32)
    wk = np.asarray(wk, dtype=np.float32)
    wv = np.asarray(wv, dtype=np.float32)
    b, H, W, dim = x.shape
    assert (b, H, W, dim) == (B, 32, 32, DIM)

    # blocked [pair, p, k, o]: wb[t, p, k, o] = w.T[k*128+p, t*128+o]
    def blocked(w):
        wt = np.ascontiguousarray(w.T).astype(QK_NP)        # [c, o]
        return np.ascontiguousarray(
            wt.reshape(4, 128, 4, 128).transpose(2, 1, 0, 3)
        )

    wqb = blocked(wq)
    wkb = blocked(wk)
    # pairs 1-3 packed: [p, {q,k}, t-1, k, o]
    wrest = np.ascontiguousarray(
        np.stack([wqb[1:4], wkb[1:4]], axis=0).transpose(2, 0, 1, 3, 4)
    )
    wvt = np.ascontiguousarray(
        wv.reshape(N_HEADS, D_HEAD, DIM).sum(axis=1).T     # [c, n]
    ).astype(QK_NP)
    ind2 = np.zeros((128, 2), dtype=QK_NP)
    ind2[np.arange(128), np.arange(128) // D_HEAD] = 1.0
    pre = np.concatenate(
        [
            wvt.reshape(4, 128, 8).transpose(1, 0, 2).reshape(128, 32),
            ind2,
            wqb[0].reshape(128, 512),
            wkb[0].reshape(128, 512),
        ],
        axis=1,
    )
    sel = np.zeros((N_HEADS, N_HEADS * 128), dtype=QK_NP)
    for n in range(N_HEADS):
        sel[n, n * 128 : (n + 1) * 128] = 1.0
    aux2 = np.zeros((8, 1026), dtype=QK_NP)
    aux2[:, 0:1024] = sel
    aux2[0:2, 1024:1026] = np.eye(2, dtype=QK_NP)

    xh = x.reshape(B, HW, DIM).astype(QK_NP)
    return [
        {
            "xt": np.ascontiguousarray(xh[i].T),           # [c, xy]
            "pre": np.ascontiguousarray(pre),
            "wrest": wrest,
            "aux2": aux2,
        }
        for i in range(N_CORES)
    ]


def kernel(x, wq, wk, wv):
    nc = _get_nc()
    in_maps = make_in_maps(x, wq, wk, wv)
    res = run_bass_kernel_spmd(nc, in_maps, list(range(N_CORES)))
    out = np.stack([res.results[i]["y"] for i in range(N_CORES)], axis=0)
    # [b, n, hw, xy] -> [b, n, h, w, x, y]; upcast fp16 -> fp32 on host
    return out.astype(np.float32).reshape(B, N_HEADS, 32, 32, 32, 32)


if __name__ == "__main__":
    rng = np.random.default_rng(0)
    x = rng.standard_normal((B, 32, 32, DIM), dtype=np.float32)
    s = 1.0 / np.sqrt(512.0)
    wq = rng.uniform(-s, s, (512, 512)).astype(np.float32)
    wk = rng.uniform(-s, s, (512, 512)).astype(np.float32)
    wv = rng.uniform(-s, s, (512, 512)).astype(np.float32)
    y = kernel(x=x, wq=wq, wk=wk, wv=wv)
    print(y.shape, y.dtype)


# revision 20
# speedup vs baseline: 1.0354x; 1.0354x over previous
"""Trainium2 Bass kernel for nn_AttnBlock (per-pixel qk attention block).

Reference computation (per batch b):
  q = x @ wq.T ; k = x @ wk.T ; v = x @ wv.T          # [H*W, 512], heads n=8, d=64
  s[n, p]    = sum_d q[p, n*64+d] * k[p, n*64+d]      # per-pixel dot product
  w[n, h, :] = softmax(s[n, h, :] * d**-0.5)          # softmax over W axis (32)
  vsum[n, p] = sum_d v[p, n*64+d]
  out[b, n, hw, xy] = w[n, hw] * vsum[n, xy]          # outer product per batch

Sharding: data-parallel over batch: core b handles batch b (8 cores, B=8).

The kernel is output-write bound: fp16 output (host upcasts; rel err ~1e-3
vs the 2e-2 gate) => 16 MB/core written + 2 MB read at the ~358 GB/s per-NC
HBM limit => ~52.7 us DMA floor. The design keeps the DMA queue busy
end-to-end: the input stream is packed into 7 DMAs (aux+pair-0 weights
merged into one "pre" tensor so x^T starts immediately), and the first
output tiles are ready shortly after the last input lands.

v6 implementation notes:
- host does layout prep only (transposes/reshapes/casts; the only
  arithmetic is the head-block sum of wv rows, O(dim^2)).
- PE warm-up: dummy matmuls (own PSUM bank) from t~0 so the tensor engine
  reaches full clock (cost model: 2.4 GHz after 3 us continuous busy)
  before the x-gated burst; spares also fill x-chunk stall gaps.
- q/k PSUM never drains: sprod = q*k reads both PSUM banks directly.
- pair 0 runs a fine-grained high-priority softmax (per-128-col chunk
  exp/reduce/recip/mult/PE-transpose) so the first 1 MB DMA starts ~5 us
  after the last x chunk; pairs 1-3 use half-granularity ops.
- production per head: PE selector-matmul broadcast of vsum (drains on
  ACT), 8 outer-product tiles [128, 1024] via DVE tensor_scalar fp16
  (4x mode, 327 ns; 2 of 8 on ACT for heads 1-7), then the head's 2 MB
  DMA (head 0: 2x 1 MB to open the chain early).
"""

import numpy as np

import concourse.bass as bass
import concourse.mybir as mybir
import concourse.tile as tile
from concourse import bacc
from concourse.bass_utils import run_bass_kernel_spmd

F32 = mybir.dt.float32
F16 = mybir.dt.float16

B, HW, DIM = 8, 1024, 512
N_HEADS, D_HEAD = 8, 64
N_CORES = 8
SCALE = float(D_HEAD) ** -0.5

QK_DT = F16
QK_NP = np.float16
OUT_DT = F16

N_WARMUP = 16  # dummy PE matmuls: ~3 us ramp + spares for x-stall gaps


def build_program(loop_iters=None):
    """loop_iters: if set, wrap the whole kernel body in a tc.For_i hardware
    loop (benchmarking only -- one NEFF executes the body N times)."""
    # Bacc (not raw Bass): its compile() runs move_matmul_waits_to_ldweights,
    # without which any matmul with >1 semaphore wait fails walrus codegen.
    nc = bacc.Bacc(None)

    xt_d = nc.declare_dram_parameter("xt", [DIM, HW], QK_DT, isOutput=False)
    # pre = aux (wv_sum + ind2) and the pair-0 wq/wk slices, one DMA
    pre_d = nc.declare_dram_parameter("pre", [128, 1058], QK_DT, isOutput=False)
    wrest_d = nc.declare_dram_parameter(
        "wrest", [128, 2, 3, 4, 128], QK_DT, isOutput=False
    )
    aux2_d = nc.declare_dram_parameter("aux2", [8, 1026], QK_DT, isOutput=False)
    y_d = nc.declare_dram_parameter("y", [N_HEADS, HW, HW], OUT_DT, isOutput=True)

    with tile.TileContext(nc) as tc:
        with (
            tc.tile_pool(name="singles", bufs=1) as singles,
            tc.tile_pool(name="sprod", bufs=2) as sprodp,
            tc.tile_pool(name="smax", bufs=2) as smaxp,
            tc.tile_pool(name="wt", bufs=2) as wtp,
            tc.tile_pool(name="bc", bufs=2) as bcp,
            tc.tile_pool(name="prod", bufs=2) as prodp,
            tc.tile_pool(name="warm_ps", bufs=1, space="PSUM") as warm_ps,
            tc.tile_pool(name="tp_ps", bufs=1, space="PSUM") as tp_ps,
            tc.tile_pool(name="qk_ps", bufs=3, space="PSUM") as qk_ps,
            tc.tile_pool(name="s_ps", bufs=1, space="PSUM") as s_ps,
            tc.tile_pool(name="v_ps", bufs=1, space="PSUM") as v_ps,
        ):
            def emit_body():
                # ---- PE warm-up tile (dummy matmuls emitted last, so they
                # have the lowest priority and only fill idle PE slots) ------
                wm = singles.tile([128, 512], QK_DT, name="wm")
                nc.gpsimd.memset(wm, 0.0)

                # ---- loads: one FIFO queue (sync HWDGE), priority order ----
                pre_sb = singles.tile([128, 1058], QK_DT)
                nc.sync.dma_start(out=pre_sb, in_=pre_d[:])
                wvt_sb = pre_sb[:, 0:32].rearrange("p (k n) -> p k n", k=4)
                ind2_sb = pre_sb[:, 32:34]
                wq0_sb = pre_sb[:, 34:546].rearrange("p (k o) -> p k o", k=4)
                wk0_sb = pre_sb[:, 546:1058].rearrange("p (k o) -> p k o", k=4)

                xT = []
                xv = xt_d[:].rearrange("(k p) xy -> p k xy", p=128)
                for ki in range(4):
                    xt_t = singles.tile([128, HW], QK_DT, name=f"xT{ki}")
                    nc.sync.dma_start(out=xt_t, in_=xv[:, ki, :])
                    xT.append(xt_t)

                aux2_sb = singles.tile([8, 1026], QK_DT)
                nc.sync.dma_start(out=aux2_sb, in_=aux2_d[:])
                sel_sb = aux2_sb[:, 0:1024]
                ident2_sb = aux2_sb[0:2, 1024:1026]

                wq_sb = singles.tile([128, 3, 4, 128], QK_DT)  # pairs 1-3
                wk_sb = singles.tile([128, 3, 4, 128], QK_DT)
                nc.sync.dma_start(out=wq_sb, in_=wrest_d[:, 0])
                nc.sync.dma_start(out=wk_sb, in_=wrest_d[:, 1])

                def wslice(w0_sb, wr_sb, ti, ki):
                    if ti == 0:
                        return w0_sb[:, ki, :]
                    return wr_sb[:, ti - 1, ki, :]

                def emit_qk_half(ti, nj):
                    """q/k matmul groups for pixel-half nj of pair ti."""
                    qps = qk_ps.tile([128, 512], F32, tag="qk", name="qps")
                    kps = qk_ps.tile([128, 512], F32, tag="qk", name="kps")
                    for ps, w0, wr in ((qps, wq0_sb, wq_sb), (kps, wk0_sb, wk_sb)):
                        for ki in range(4):
                            nc.tensor.matmul(
                                ps,
                                wslice(w0, wr, ti, ki),
                                xT[ki][:, nj * 512 : (nj + 1) * 512],
                                start=(ki == 0),
                                stop=(ki == 3),
                            )
                    return qps, kps

                def emit_scores_half(qps, kps):
                    """sprod straight from the q/k PSUM banks + score matmul."""
                    sprod = sprodp.tile([128, 512], QK_DT, tag="sp")
                    nc.vector.tensor_tensor(
                        out=sprod, in0=qps, in1=kps, op=mybir.AluOpType.mult,
                    )
                    sps = s_ps.tile([2, 512], F32, tag="s")
                    nc.tensor.matmul(sps, ind2_sb, sprod, start=True, stop=True)
                    return sps

                def emit_softmax_chunk(sps, cloc, ncol):
                    """exp/reduce/recip/mult over sps cols [cloc, cloc+ncol).
                    Returns the w chunk tile [2, ncol] fp16."""
                    csl = slice(cloc, cloc + ncol)
                    e_h = smaxp.tile([2, ncol], QK_DT, tag=f"e{cloc}_{ncol}")
                    nc.scalar.activation(
                        out=e_h, in_=sps[:, csl],
                        func=mybir.ActivationFunctionType.Exp,
                        scale=SCALE,
                    )
                    nh = ncol // 32
                    denom = smaxp.tile([2, nh], QK_DT, tag=f"d{cloc}_{ncol}")
                    with nc.allow_low_precision(reason="fp16 softmax denom"):
                        nc.vector.tensor_reduce(
                            out=denom,
                            in_=e_h.rearrange("p (h w) -> p h w", w=32),
                            axis=mybir.AxisListType.X,
                            op=mybir.AluOpType.add,
                        )
                        rden = smaxp.tile([2, nh], QK_DT, tag=f"r{cloc}_{ncol}")
                        nc.vector.reciprocal(rden, denom)
                    w_h = smaxp.tile([2, ncol], QK_DT, tag=f"w{cloc}_{ncol}")
                    rden_b = bass.AP(
                        tensor=rden.tensor, offset=rden.offset,
                        ap=[*rden.ap, [0, 32]],
                    )
                    nc.vector.tensor_tensor(
                        out=w_h.rearrange("p (h w) -> p h w", w=32),
                        in0=e_h.rearrange("p (h w) -> p h w", w=32),
                        in1=rden_b,
                        op=mybir.AluOpType.mult,
                    )
                    return w_h

                def emit_bcast(head, bcast_t=None):
                    """vsum row -> all partitions via PE selector matmul;
                    PSUM drains on ACT."""
                    if bcast_t is None:
                        bcast_t = bcp.tile([128, HW], QK_DT, tag="bc", name="bc")
                    for nj in range(2):
                        bps = qk_ps.tile([128, 512], F32, tag="qk", name="bps")
                        nc.tensor.matmul(
                            bps,
                            sel_sb[:, head * 128 : (head + 1) * 128],
                            vsum_sb[:, nj * 512 : (nj + 1) * 512],
                            start=True, stop=True,
                        )
                        nc.scalar.copy(bcast_t[:, nj * 512 : (nj + 1) * 512], bps)
                    return bcast_t

                def dma_rows(head, prod_t, j, c0, c1):
                    nc.sync.dma_start(
                        out=y_d[head : head + 1].rearrange(
                            "n (c p) xy -> p n c xy", p=128
                        )[:, :, c0:c1, :],
                        in_=prod_t[:, j : j + 1, c0:c1, :],
                    )

                # ---- pair 0 first: its q/k + scores get high priority so
                # the first output DMA lands as early as possible; vsum sits
                # between pair-0 h0 and h1 on the PE. All pairs share the
                # same half-granularity softmax/production pipeline.
                with tc.high_priority():
                    qk00 = emit_qk_half(0, 0)

                vps = v_ps.tile([N_HEADS, HW], F32)
                vsum_sb = singles.tile([N_HEADS, HW], QK_DT)
                for nj in range(2):
                    for ki in range(4):
                        nc.tensor.matmul(
                            vps[:, nj * 512 : (nj + 1) * 512],
                            wvt_sb[:, ki, :],
                            xT[ki][:, nj * 512 : (nj + 1) * 512],
                            start=(ki == 0),
                            stop=(ki == 3),
                        )

                with tc.high_priority():
                    sps00 = emit_scores_half(*qk00)
                    w_h00 = emit_softmax_chunk(sps00, 0, 512)

                # vsum -> fp16 halves (ACT; gates the selector broadcast)
                for nj in range(2):
                    nc.scalar.copy(
                        vsum_sb[:, nj * 512 : (nj + 1) * 512],
                        vps[:, nj * 512 : (nj + 1) * 512],
                    )

                for ti in range(4):
                    wt_sb = wtp.tile([128, 8, 2], F32, tag="wt", name="wt")
                    tp = tp_ps.tile([128, 16], QK_DT, tag="tp", name="tp")

                    def transpose4(w_h, cbase):
                        for cr in range(4):
                            nc.tensor.transpose(
                                tp[:, (cbase + cr) * 2 : (cbase + cr + 1) * 2],
                                w_h[:, cr * 128 : (cr + 1) * 128],
                                ident2_sb,
                            )

                    def drain_wt(c0, c1):
                        nc.vector.tensor_copy(
                            wt_sb[:, c0:c1, :],
                            tp[:, c0 * 2 : c1 * 2].rearrange(
                                "p (c n) -> p c n", c=c1 - c0
                            ),
                        )

                    prod_t = prodp.tile([128, 2, 8, HW], OUT_DT, tag="pr", name="pr")

                    def produce(j, cj, bcast_t):
                        head = 2 * ti + j
                        if cj in (2, 5) and head > 0:
                            nc.scalar.activation(
                                out=prod_t[:, j, cj, :], in_=bcast_t,
                                func=mybir.ActivationFunctionType.Copy,
                                scale=wt_sb[:, cj, j : j + 1],
                            )
                        else:
                            nc.vector.tensor_scalar_mul(
                                prod_t[:, j, cj, :], bcast_t,
                                wt_sb[:, cj, j : j + 1],
                            )

                    if ti == 0:
                        with tc.high_priority():
                            qps, kps = emit_qk_half(0, 1)
                            sps = emit_scores_half(qps, kps)
                            w_h01 = emit_softmax_chunk(sps, 0, 512)
                            transpose4(w_h00, 0)
                            transpose4(w_h01, 4)
                            drain_wt(0, 8)
                            for j in range(2):
                                bcast_t = emit_bcast(j)
                                for cj in range(8):
                                    produce(j, cj, bcast_t)
                                    if j == 0 and cj == 3:
                                        dma_rows(0, prod_t, 0, 0, 4)
                                if j == 0:
                                    dma_rows(0, prod_t, 0, 4, 8)
                                else:
                                    dma_rows(1, prod_t, 1, 0, 8)
                        continue

                    w_halves = []
                    for nj in range(2):
                        qps, kps = emit_qk_half(ti, nj)
                        sps = emit_scores_half(qps, kps)
                        w_halves.append(emit_softmax_chunk(sps, 0, 512))
                    transpose4(w_halves[0], 0)
                    transpose4(w_halves[1], 4)
                    drain_wt(0, 8)
                    for j in range(2):
                        head = 2 * ti + j
                        bcast_t = emit_bcast(head)
                        for cj in range(8):
                            produce(j, cj, bcast_t)
                        dma_rows(head, prod_t, j, 0, 8)

                # PE warm-up dummies: emitted last => lowest priority, they
                # only run when no real matmul is ready (t~0 and x-stalls)
                for wi in range(N_WARMUP):
                    wps = warm_ps.tile([128, 512], F32, tag="w")
                    nc.tensor.matmul(
                        wps, wm[:, 0:128], wm, start=True, stop=True,
                    )

            if loop_iters:
                with tc.For_i(0, loop_iters, 1):
                    emit_body()
            else:
                emit_body()

    nc.compile()
    return nc


_NC_CACHE = None


def _get_nc():
    global _NC_CACHE
    if _NC_CACHE is None:
        _NC_CACHE = build_program()
    return _NC_CACHE


def make_in_maps(x, wq, wk, wv):
    """Host-side input prep: dtype casts and layout transforms only (transpose,
    reshape, head-block sum of wv -- no x-dependent compute beyond layout),
    plus per-core batch sharding."""
    x = np.ascontiguousarray(np.asarray(x, dtype=np.float32))
    wq = np.asarray(wq, dtype=np.float32)
    wk = np.asarray(wk, dtype=np.float32)
    wv = np.asarray(wv, dtype=np.float32)
    b, H, W, dim = x.shape
    assert (b, H, W, dim) == (B, 32, 32, DIM)

    # blocked [pair, p, k, o]: wb[t, p, k, o] = w.T[k*128+p, t*128+o]
    def blocked(w):
        wt = np.ascontiguousarray(w.T).astype(QK_NP)        # [c, o]
        return np.ascontiguousarray(
            wt.reshape(4, 128, 4, 128).transpose(2, 1, 0, 3)
        )

    wqb = blocked(wq)
    wkb = blocked(wk)
    # pairs 1-3 packed: [p, {q,k}, t-1, k, o]
    wrest = np.ascontiguousarray(
        np.stack([wqb[1:4], wkb[1:4]], axis=0).transpose(2, 0, 1, 3, 4)
    )
    wvt = np.ascontiguousarray(
        wv.reshape(N_HEADS, D_HEAD, DIM).sum(axis=1).T     # [c, n]
    ).astype(QK_NP)
    ind2 = np.zeros((128, 2), dtype=QK_NP)
    ind2[np.arange(128), np.arange(128) // D_HEAD] = 1.0
    pre = np.concatenate(
        [
            wvt.reshape(4, 128, 8).transpose(1, 0, 2).reshape(128, 32),
            ind2,
            wqb[0].reshape(128, 512),
            wkb[0].reshape(128, 512),
        ],
        axis=1,
    )
    sel = np.zeros((N_HEADS, N_HEADS * 128), dtype=QK_NP)
    for n in range(N_HEADS):
        sel[n, n * 128 : (n + 1) * 128] = 1.0
    aux2 = np.zeros((8, 1026), dtype=QK_NP)
    aux2[:, 0:1024] = sel
    aux2[0:2, 1024:1026] = np.eye(2, dtype=QK_NP)

    xh = x.reshape(B, HW, DIM).astype(QK_NP)
    return [
        {
            "xt": np.ascontiguousarray(xh[i].T),           # [c, xy]
            "pre": np.ascontiguousarray(pre),
            "wrest": wrest,
            "aux2": aux2,
        }
        for i in range(N_CORES)
    ]


def kernel(x, wq, wk, wv):
    nc = _get_nc()
    in_maps = make_in_maps(x, wq, wk, wv)
    res = run_bass_kernel_spmd(nc, in_maps, list(range(N_CORES)))
    out = np.stack([res.results[i]["y"] for i in range(N_CORES)], axis=0)
    # [b, n, hw, xy] -> [b, n, h, w, x, y]; upcast fp16 -> fp32 on host
    return out.astype(np.float32).reshape(B, N_HEADS, 32, 32, 32, 32)


if __name__ == "__main__":
    rng = np.random.default_rng(0)
    x = rng.standard_normal((B, 32, 32, DIM), dtype=np.float32)
    s = 1.0 / np.sqrt(512.0)
    wq = rng.uniform(-s, s, (512, 512)).astype(np.float32)
    wk = rng.uniform(-s, s, (512, 512)).astype(np.float32)
    wv = rng.uniform(-s, s, (512, 512)).astype(np.float32)
    y = kernel(x=x, wq=wq, wk=wk, wv=wv)
    print(y.shape, y.dtype)
